# revision 1
# baseline (speedup 1.0000x reference)
"""HCNN (known-U) recurrence kernel for 8 Trainium2 NeuronCores.

Model (see reference): 80 sequential steps of
    state' = tanh(cat(post_state, u)) @ A            A: (2112, 2048) fp32
with teacher forcing post_state[:, :128] = y during the 64 past steps,
outputs = 64 past errors then 16 forecasts (first 128 state components).

Strategy
--------
Data-parallel over batch: 256 = 8 cores x 32. Each core runs the full
recurrence for its batch slice; no collectives.

Per-core per-step matmul x @ A with batch M=32 would waste 3/4 of the
128-wide PE array, so the A columns are split into 4 interleaved groups
and computed by 4 concurrent column-tiled matmuls (tile_position=(0,32j))
sharing the array. Data is fp16 (single pass): the teacher-forced
recurrence is strongly contractive, emulation shows end-to-end output
error ~1.5e-4 relative vs the fp32 reference.

Column interleave: state column s lives in col-group j=(s//32)%4 at free
offset 32*(s//128) + s%32. With that mapping the (128, 512) psum holding
state' (batch on partitions within each 32-group) turns into the next
step's stationary operand layout via a single DVE 32x32 block-transpose:
block (j, m') lands at partitions [32j:32j+32] of k-tile m' -- exactly
where matmul round m' reads it. ACT applies tanh (psum -> fp16 SBUF),
DVE transposes, PE consumes; y/u/init contributions are pre-tanh'ed and
pre-transposed on the host, so past-step rounds k=0 (y) and k=16 (u)
have no dependency on the transpose and hide its latency.
"""

import sys

for _p in ("/opt/trn_rl_repo", "/root/.axon_site/_ro/trn_rl_repo"):
    if _p not in sys.path:
        sys.path.insert(0, _p)

import numpy as np

N_STATE = 2048
N_U = 64
N_Y = 128
PAST = 64
FORE = 16
BATCH = 256
T = PAST + FORE          # 80 total steps; only 79 matmul steps needed
NSTEP = T - 1            # step t computes state_{t+1}; state_80 is unused
NK = 17                  # contraction tiles: 16 x 128 state + 1 x (64 u + 64 pad)
KDIM = NK * 128          # 2176 padded contraction size
N_CORES = 8
B = BATCH // N_CORES     # 32 per core


def _build_program():
    import concourse.bass as bass
    import concourse.tile as tile
    from concourse import mybir

    F32 = mybir.dt.float32
    F16 = mybir.dt.float16

    nc = bass.Bass("TRN2", target_bir_lowering=False, debug=False,
                   num_devices=N_CORES)

    A_ext = nc.declare_dram_parameter("A_re", [KDIM, 4, 512], F16, isOutput=False)
    ytanhT_ext = nc.declare_dram_parameter("ytanhT", [128, PAST * B], F16, isOutput=False)
    utanhT_ext = nc.declare_dram_parameter("utanhT", [128, NSTEP * B], F16, isOutput=False)
    ywrap_ext = nc.declare_dram_parameter("ywrap", [128, (PAST - 1) * B], F32, isOutput=False)
    initxT_ext = nc.declare_dram_parameter("initxT", [128, 512], F16, isOutput=False)
    out_ext = nc.declare_dram_parameter("outbuf", [128, NSTEP * B], F32, isOutput=True)

    with tile.TileContext(nc) as tc:
        with tc.tile_pool(name="const", bufs=1) as cpool, \
             tc.tile_pool(name="xbuf", bufs=2) as xpool, \
             tc.tile_pool(name="th", bufs=2) as thpool, \
             tc.tile_pool(name="psum", bufs=2, space="PSUM") as pspool:

            A_sb = cpool.tile([128, NK * 2048], F16, tag="A")
            for k in range(NK):
                nc.sync.dma_start(out=A_sb[:, 2048 * k:2048 * (k + 1)],
                                  in_=A_ext[128 * k:128 * (k + 1), :, :])
            ytanhT = cpool.tile([128, PAST * B], F16, tag="yt")
            nc.sync.dma_start(out=ytanhT[:], in_=ytanhT_ext[:])
            utanhT = cpool.tile([128, NSTEP * B], F16, tag="ut")
            nc.sync.dma_start(out=utanhT[:], in_=utanhT_ext[:])
            ywrap = cpool.tile([128, (PAST - 1) * B], F32, tag="yw")
            nc.sync.dma_start(out=ywrap[:], in_=ywrap_ext[:])
            outbuf = cpool.tile([128, NSTEP * B], F32, tag="ob")

            xlo = xpool.tile([128, 256], F16, tag="xlo")
            xhi = xpool.tile([128, 256], F16, tag="xhi")
            nc.sync.dma_start(out=xlo[:], in_=initxT_ext[:, 0:256])
            nc.sync.dma_start(out=xhi[:], in_=initxT_ext[:, 256:512])

            def lhs_for(t, k, lo, hi):
                if k == 0:
                    if t < PAST:
                        return ytanhT[:, B * t:B * (t + 1)]
                    return lo[:, 0:32]
                if k == 16:
                    return utanhT[:, B * t:B * (t + 1)]
                if k < 8:
                    return lo[:, 32 * k:32 * (k + 1)]
                return hi[:, 32 * (k - 8):32 * (k - 7)]

            for t in range(NSTEP):
                ps = pspool.tile([128, 512], F32, tag="ps")
                # k emission order: y and u tiles first (no transpose dep),
                # then the state tiles as the transposes complete.
                korder = [0, 16] + list(range(1, 16))
                for idx, k in enumerate(korder):
                    lhsT = lhs_for(t, k, xlo, xhi)
                    start = idx == 0
                    stop = idx == len(korder) - 1
                    for j in range(4):
                        nc.tensor.matmul(
                            ps[32 * j:32 * (j + 1), :],
                            lhsT,
                            A_sb[:, 2048 * k + 512 * j:2048 * k + 512 * (j + 1)],
                            start=start, stop=stop,
                            tile_position=(0, 32 * j),
                        )

                # output slot t+1 from this psum (expectation = cols 0:128 of
                # state', living in psum[:, 0:32] across all partition groups)
                if t + 1 < PAST:
                    nc.vector.tensor_sub(outbuf[:, B * t:B * (t + 1)],
                                         ps[:, 0:32],
                                         ywrap[:, B * t:B * (t + 1)])
                else:
                    nc.vector.tensor_copy(outbuf[:, B * t:B * (t + 1)],
                                          ps[:, 0:32])

                if t < NSTEP - 1:
                    th_lo = thpool.tile([128, 256], F16, tag="thlo")
                    nc.scalar.activation(th_lo[:], ps[:, 0:256],
                                         mybir.ActivationFunctionType.Tanh)
                    nlo = xpool.tile([128, 256], F16, tag="xlo")
                    nc.vector.transpose(nlo[:], th_lo[:])
                    th_hi = thpool.tile([128, 256], F16, tag="thhi")
                    nc.scalar.activation(th_hi[:], ps[:, 256:512],
                                         mybir.ActivationFunctionType.Tanh)
                    nhi = xpool.tile([128, 256], F16, tag="xhi")
                    nc.vector.transpose(nhi[:], th_hi[:])
                    xlo, xhi = nlo, nhi

            nc.sync.dma_start(out=out_ext[:], in_=outbuf[:])

    _split_multi_waits(nc)
    return nc


def _split_multi_waits(nc):
    """This walrus build accepts at most one sem wait per instruction; Tile
    sometimes emits more. Hoist extras onto nops inserted just before the
    instruction in the same engine stream."""
    from concourse import mybir

    n = 0
    for f in nc.m.functions:
        for b in f.blocks:
            insts = b.instructions
            out = []
            changed = False
            for ins in insts:
                si = ins.sync_info
                if si is not None and len(si.on_wait) > 1:
                    waits = list(si.on_wait)
                    for w in waits[:-1]:
                        n += 1
                        out.append(mybir.InstNoOp(
                            name=f"I-waitsplit-{n}",
                            engine=ins.engine,
                            ins=[], outs=[],
                            bass_nofuse=True,
                            sync_info=mybir.SyncInfo(on_wait=[w], on_update=[]),
                        ))
                    ins.sync_info = mybir.SyncInfo(
                        on_wait=[waits[-1]], on_update=list(si.on_update))
                    changed = True
                out.append(ins)
            if changed:
                b.instructions = out


def _host_inputs(U, Y, A, init_state):
    """Build the per-core input maps (all pre-tanh / pre-transpose work)."""
    A = np.asarray(A, np.float32)
    U = np.asarray(U, np.float32)
    Y = np.asarray(Y, np.float32)
    init_state = np.asarray(init_state, np.float32)

    A_pad = np.zeros((KDIM, N_STATE), np.float16)
    A_pad[:N_STATE + N_U] = A.astype(np.float16)
    # column interleave: col s -> (j=(s//32)%4, free 32*(s//128)+s%32)
    A_re = np.ascontiguousarray(
        A_pad.reshape(KDIM, 16, 4, 32).transpose(0, 2, 1, 3).reshape(KDIM, 4, 512))

    init_tanh = np.tanh(init_state[0]).astype(np.float16)          # (2048,)
    initxT = np.ascontiguousarray(
        np.broadcast_to(init_tanh.reshape(16, 128).T[:, None, :].transpose(0, 2, 1),
                        (128, 16, 32)).reshape(128, 512))

    ytanh = np.tanh(Y).astype(np.float16)                          # (64, 256, 128)
    utanh = np.tanh(U[:NSTEP]).astype(np.float16)                  # (79, 256, 64)

    in_maps = []
    for c in range(N_CORES):
        b0 = c * B
        yt = np.ascontiguousarray(
            ytanh[:, b0:b0 + B, :].transpose(0, 2, 1)              # (64, 128, 32)
            .transpose(1, 0, 2).reshape(128, PAST * B))
        ut = np.zeros((128, NSTEP * B), np.float16)
        ut[:N_U] = (utanh[:, b0:b0 + B, :].transpose(0, 2, 1)      # (79, 64, 32)
                    .transpose(1, 0, 2).reshape(N_U, NSTEP * B))
        # ywrap slot s (=1..63) at cols 32*(s-1): rows 32j+b = Y[s, b0+b, 32j+cc]
        yw = (Y[1:PAST, b0:b0 + B, :].reshape(PAST - 1, B, 4, 32)
              .transpose(0, 2, 1, 3)                               # (63, 4, 32b, 32cc)
              .reshape(PAST - 1, 128, 32)
              .transpose(1, 0, 2).reshape(128, (PAST - 1) * B))
        in_maps.append({
            "A_re": A_re,
            "ytanhT": yt,
            "utanhT": np.ascontiguousarray(ut),
            "ywrap": np.ascontiguousarray(yw.astype(np.float32)),
            "initxT": initxT,
        })
    return in_maps


def kernel(U, Y, A, init_state):
    from concourse.bass_utils import run_bass_kernel_spmd

    nc = _build_program()
    in_maps = _host_inputs(U, Y, A, init_state)
    res = run_bass_kernel_spmd(nc, in_maps, list(range(N_CORES)))

    out = np.empty((T, BATCH, N_Y), np.float32)
    # slot 0: err for t=0 is pure host math (state_0 = broadcast init_state)
    out[0] = np.asarray(init_state, np.float32)[0, :N_Y][None, :] - np.asarray(Y, np.float32)[0]
    for c in range(N_CORES):
        b0 = c * B
        ob = res.results[c]["outbuf"]                              # (128, 79*32)
        # [32j+b, 32t+cc] = out[t+1, b0+b, 32j+cc]
        ob4 = ob.reshape(4, 32, NSTEP, 32)                         # (j, b, t, cc)
        out[1:, b0:b0 + B, :] = ob4.transpose(2, 1, 0, 3).reshape(NSTEP, B, N_Y)
    return out


if __name__ == "__main__":
    rng = np.random.default_rng(0)
    U = rng.standard_normal((T, BATCH, N_U)).astype(np.float32)
    Y = rng.standard_normal((PAST, BATCH, N_Y)).astype(np.float32)
    A = (rng.standard_normal((N_STATE + N_U, N_STATE)) * 0.02).astype(np.float32)
    init = rng.standard_normal((1, N_STATE)).astype(np.float32)
    o = kernel(U=U, Y=Y, A=A, init_state=init)
    print("kernel out:", o.shape, o.dtype)



# revision 7
# speedup vs baseline: 1.0988x; 1.0988x over previous
"""HCNN (known-U) recurrence kernel for 8 Trainium2 NeuronCores.

Model (see reference): 80 sequential steps of
    state' = tanh(cat(post_state, u)) @ A            A: (2112, 2048) fp32
with teacher forcing post_state[:, :128] = y during the 64 past steps,
outputs = 64 past errors then 16 forecasts (first 128 state components).

Strategy
--------
Data-parallel over batch: 256 = 8 cores x 32. Each core runs the full
recurrence for its batch slice; no collectives.

Per-core per-step matmul x @ A with batch M=32 would waste 3/4 of the
128-wide PE array, so the A columns are split into 4 interleaved groups
and computed by 4 concurrent column-tiled matmuls (tile_position=(0,32j))
sharing the array. Data is fp16 (single pass): the teacher-forced
recurrence is strongly contractive; end-to-end output error ~1.5e-4
relative vs the fp32 reference.

Column interleave: state column s lives in col-group j=(s//32)%4 at free
offset 32*(s//128) + s%32. The (128, 512) psum holding state' (batch on
partitions within each 32-group) turns into the next step's stationary
operand layout via DVE 32x32 block-transposes: block (j, m') lands at
partitions [32j:32j+32] of k-tile m' -- exactly where matmul round m'
reads it.

Pipelining (the point of this version): each step's 17 k-rounds are
split into lo (psum cols 0:256 = k-tiles 0..7 of the next x) and hi
(cols 256:512 = k-tiles 8..15) accumulation groups. The lo group stops
half way through the step, so ACT tanh + DVE transpose of the lo chunks
overlap the hi matmuls, and the hi chunks' tanh/transpose overlaps the
next step's early rounds (which consume y/u/lo-chunk operands first).
The PE never waits on the full tanh->transpose chain. A tiles are
DMA-streamed in first-use order so step 0 starts as soon as tile k=0
lands; outputs are DMA'd out in slices as they are produced.
"""

import sys

for _p in ("/opt/trn_rl_repo", "/root/.axon_site/_ro/trn_rl_repo"):
    if _p not in sys.path:
        sys.path.insert(0, _p)

import numpy as np

N_STATE = 2048
N_U = 64
N_Y = 128
PAST = 64
FORE = 16
BATCH = 256
T = PAST + FORE          # 80 total steps; only 79 matmul steps needed
NSTEP = T - 1            # step t computes state_{t+1}; state_80 is unused
NK = 17                  # contraction tiles: 16 x 128 state + 1 x (64 u + 64 pad)
KDIM = NK * 128          # 2176 padded contraction size
N_CORES = 8
B = BATCH // N_CORES     # 32 per core
OUT_CHUNK = 10           # output DMA granularity in steps


def _build_program():
    import concourse.bass as bass
    import concourse.tile as tile
    from concourse import mybir

    F32 = mybir.dt.float32
    F16 = mybir.dt.float16

    nc = bass.Bass("TRN2", target_bir_lowering=False, debug=False,
                   num_devices=N_CORES)

    A_ext = nc.declare_dram_parameter("A_re", [KDIM, 4, 512], F16, isOutput=False)
    ytanhT_ext = nc.declare_dram_parameter("ytanhT", [128, PAST * B], F16, isOutput=False)
    utanhT_ext = nc.declare_dram_parameter("utanhT", [128, NSTEP * B], F16, isOutput=False)
    ywrap_ext = nc.declare_dram_parameter("ywrap", [128, (PAST - 1) * B], F32, isOutput=False)
    initxT_ext = nc.declare_dram_parameter("initxT", [128, 512], F16, isOutput=False)
    out_ext = nc.declare_dram_parameter("outbuf", [128, NSTEP * B], F32, isOutput=True)

    # k emission order: y and u tiles first (static operands, no transpose
    # dep), then state tiles 1..7 (lo chunks, ready early), then 8..15 (hi
    # chunks, arriving while rounds 1..7 stream).
    korder = [0, 16] + list(range(1, 16))

    with tile.TileContext(nc) as tc:
        with tc.tile_pool(name="const", bufs=1) as cpool, \
             tc.tile_pool(name="xbuf", bufs=2) as xpool, \
             tc.tile_pool(name="th", bufs=2) as thpool, \
             tc.tile_pool(name="psum", bufs=2, space="PSUM") as pspool:

            ytanhT = cpool.tile([128, PAST * B], F16, tag="yt")
            nc.sync.dma_start(out=ytanhT[:], in_=ytanhT_ext[:])
            xT = xpool.tile([128, 512], F16, tag="xT")
            nc.sync.dma_start(out=xT[:], in_=initxT_ext[:])
            utanhT = cpool.tile([128, NSTEP * B], F16, tag="ut")
            nc.sync.dma_start(out=utanhT[:], in_=utanhT_ext[:])

            # A k-tiles as separate tiles, DMA'd in first-use order so each
            # round of step 0 waits only for its own tile.
            A_sb = [None] * NK
            for k in korder:
                A_sb[k] = cpool.tile([128, 2048], F16, tag=f"A{k}", name=f"A{k}")
                nc.sync.dma_start(out=A_sb[k][:],
                                  in_=A_ext[128 * k:128 * (k + 1), :, :])

            ywrap = cpool.tile([128, (PAST - 1) * B], F32, tag="yw")
            nc.sync.dma_start(out=ywrap[:], in_=ywrap_ext[:])
            outbuf = cpool.tile([128, NSTEP * B], F32, tag="ob")

            def lhs_for(t, k, x):
                if k == 0:
                    if t < PAST:
                        return ytanhT[:, B * t:B * (t + 1)]
                    return x[:, 0:32]
                if k == 16:
                    return utanhT[:, B * t:B * (t + 1)]
                return x[:, 32 * k:32 * (k + 1)]

            out_done = 0
            for t in range(NSTEP):
                ps_lo = pspool.tile([128, 256], F32, tag="plo")
                ps_hi = pspool.tile([128, 256], F32, tag="phi")
                if t < NSTEP - 1:
                    nxT = xpool.tile([128, 512], F16, tag="xT", name="nxT")

                # the last step only needs output cols 0:128 (= lo cols 0:32
                # of each quadrant); its hi half is never read.
                halves = ((0, ps_lo), (1, ps_hi)) if t < NSTEP - 1 else ((0, ps_lo),)
                for half, ps in halves:
                    c0 = 256 * half
                    for idx, k in enumerate(korder):
                        lhsT = lhs_for(t, k, xT)
                        start = idx == 0
                        stop = idx == len(korder) - 1
                        for j in range(4):
                            nc.tensor.matmul(
                                ps[32 * j:32 * (j + 1), :],
                                lhsT,
                                A_sb[k][:, 512 * j + c0:512 * j + c0 + 256],
                                start=start, stop=stop,
                                tile_position=(0, 32 * j),
                            )
                    if t == NSTEP - 1:
                        continue  # last state: only the output copy is needed
                    # tanh + 32x32 block transpose of this half's chunks into
                    # the next step's stationary operand tile. Chunk 0 is only
                    # needed once teacher forcing ends (t >= PAST - 1).
                    if half == 0:
                        lo0 = 32 if t < PAST - 1 else 0
                        for a, b_ in ((lo0, 128), (128, 256)):
                            th = thpool.tile([128, b_ - a], F16, tag=f"th{a}")
                            nc.scalar.activation(th[:], ps[:, a:b_],
                                                 mybir.ActivationFunctionType.Tanh)
                            nc.vector.transpose(nxT[:, a:b_], th[:])
                    else:
                        for a, b_ in ((256, 384), (384, 512)):
                            th = thpool.tile([128, b_ - a], F16, tag=f"th{a}")
                            nc.scalar.activation(th[:], ps[:, a - 256:b_ - 256],
                                                 mybir.ActivationFunctionType.Tanh)
                            nc.vector.transpose(nxT[:, a:b_], th[:])

                # output slot t (row t+1): expectation = psum cols 0:32 of
                # every partition group. GPSIMD can't read PSUM, so this rides
                # DVE, issued after the latency-critical transposes.
                if t + 1 < PAST:
                    nc.vector.tensor_sub(outbuf[:, B * t:B * (t + 1)],
                                         ps_lo[:, 0:32],
                                         ywrap[:, B * t:B * (t + 1)])
                else:
                    nc.vector.tensor_copy(outbuf[:, B * t:B * (t + 1)],
                                          ps_lo[:, 0:32])
                if t < NSTEP - 1:
                    xT = nxT

                # stream finished output slices out while compute continues
                if (t + 1) % OUT_CHUNK == 0:
                    nc.sync.dma_start(
                        out=out_ext[:, B * out_done:B * (t + 1)],
                        in_=outbuf[:, B * out_done:B * (t + 1)])
                    out_done = t + 1

            if out_done < NSTEP:
                nc.sync.dma_start(out=out_ext[:, B * out_done:],
                                  in_=outbuf[:, B * out_done:])

    _split_multi_waits(nc)
    return nc


def _split_multi_waits(nc):
    """This walrus build accepts at most one sem wait per instruction; Tile
    sometimes emits more. Hoist extras onto nops inserted just before the
    instruction in the same engine stream."""
    from concourse import mybir

    n = 0
    for f in nc.m.functions:
        for b in f.blocks:
            insts = b.instructions
            out = []
            changed = False
            for ins in insts:
                si = ins.sync_info
                if si is not None and len(si.on_wait) > 1:
                    waits = list(si.on_wait)
                    for w in waits[:-1]:
                        n += 1
                        out.append(mybir.InstNoOp(
                            name=f"I-waitsplit-{n}",
                            engine=ins.engine,
                            ins=[], outs=[],
                            bass_nofuse=True,
                            sync_info=mybir.SyncInfo(on_wait=[w], on_update=[]),
                        ))
                    ins.sync_info = mybir.SyncInfo(
                        on_wait=[waits[-1]], on_update=list(si.on_update))
                    changed = True
                out.append(ins)
            if changed:
                b.instructions = out


def _host_inputs(U, Y, A, init_state):
    """Build the per-core input maps (all pre-tanh / pre-transpose work)."""
    A = np.asarray(A, np.float32)
    U = np.asarray(U, np.float32)
    Y = np.asarray(Y, np.float32)
    init_state = np.asarray(init_state, np.float32)

    A_pad = np.zeros((KDIM, N_STATE), np.float16)
    A_pad[:N_STATE + N_U] = A.astype(np.float16)
    # column interleave: col s -> (j=(s//32)%4, free 32*(s//128)+s%32)
    A_re = np.ascontiguousarray(
        A_pad.reshape(KDIM, 16, 4, 32).transpose(0, 2, 1, 3).reshape(KDIM, 4, 512))

    init_tanh = np.tanh(init_state[0]).astype(np.float16)          # (2048,)
    initxT = np.ascontiguousarray(
        np.broadcast_to(init_tanh.reshape(16, 128).T[:, None, :].transpose(0, 2, 1),
                        (128, 16, 32)).reshape(128, 512))

    ytanh = np.tanh(Y).astype(np.float16)                          # (64, 256, 128)
    utanh = np.tanh(U[:NSTEP]).astype(np.float16)                  # (79, 256, 64)

    in_maps = []
    for c in range(N_CORES):
        b0 = c * B
        yt = np.ascontiguousarray(
            ytanh[:, b0:b0 + B, :].transpose(0, 2, 1)              # (64, 128, 32)
            .transpose(1, 0, 2).reshape(128, PAST * B))
        ut = np.zeros((128, NSTEP * B), np.float16)
        ut[:N_U] = (utanh[:, b0:b0 + B, :].transpose(0, 2, 1)      # (79, 64, 32)
                    .transpose(1, 0, 2).reshape(N_U, NSTEP * B))
        # ywrap slot s (=1..63) at cols 32*(s-1): rows 32j+b = Y[s, b0+b, 32j+cc]
        yw = (Y[1:PAST, b0:b0 + B, :].reshape(PAST - 1, B, 4, 32)
              .transpose(0, 2, 1, 3)                               # (63, 4, 32b, 32cc)
              .reshape(PAST - 1, 128, 32)
              .transpose(1, 0, 2).reshape(128, (PAST - 1) * B))
        in_maps.append({
            "A_re": A_re,
            "ytanhT": yt,
            "utanhT": np.ascontiguousarray(ut),
            "ywrap": np.ascontiguousarray(yw.astype(np.float32)),
            "initxT": initxT,
        })
    return in_maps


def kernel(U, Y, A, init_state):
    from concourse.bass_utils import run_bass_kernel_spmd

    nc = _build_program()
    in_maps = _host_inputs(U, Y, A, init_state)
    res = run_bass_kernel_spmd(nc, in_maps, list(range(N_CORES)))

    out = np.empty((T, BATCH, N_Y), np.float32)
    # slot 0: err for t=0 is pure host math (state_0 = broadcast init_state)
    out[0] = np.asarray(init_state, np.float32)[0, :N_Y][None, :] - np.asarray(Y, np.float32)[0]
    for c in range(N_CORES):
        b0 = c * B
        ob = res.results[c]["outbuf"]                              # (128, 79*32)
        # [32j+b, 32t+cc] = out[t+1, b0+b, 32j+cc]
        ob4 = ob.reshape(4, 32, NSTEP, 32)                         # (j, b, t, cc)
        out[1:, b0:b0 + B, :] = ob4.transpose(2, 1, 0, 3).reshape(NSTEP, B, N_Y)
    return out


if __name__ == "__main__":
    rng = np.random.default_rng(0)
    U = rng.standard_normal((T, BATCH, N_U)).astype(np.float32)
    Y = rng.standard_normal((PAST, BATCH, N_Y)).astype(np.float32)
    A = (rng.standard_normal((N_STATE + N_U, N_STATE)) * 0.02).astype(np.float32)
    init = rng.standard_normal((1, N_STATE)).astype(np.float32)
    o = kernel(U=U, Y=Y, A=A, init_state=init)
    print("kernel out:", o.shape, o.dtype)


# revision 11
# speedup vs baseline: 1.1029x; 1.0037x over previous
"""HCNN (known-U) recurrence kernel for 8 Trainium2 NeuronCores.

Model (see reference): 80 sequential steps of
    state' = tanh(cat(post_state, u)) @ A            A: (2112, 2048) fp32
with teacher forcing post_state[:, :128] = y during the 64 past steps,
outputs = 64 past errors then 16 forecasts (first 128 state components).

Strategy
--------
Data-parallel over batch: 256 = 8 cores x 32. Each core runs the full
recurrence for its batch slice; no collectives.

Per-core per-step matmul x @ A with batch M=32 would waste 3/4 of the
128-wide PE array, so the A columns are split into 4 interleaved groups
and computed by 4 concurrent column-tiled matmuls (tile_position=(0,32j))
sharing the array. Data is fp16 (single pass): the teacher-forced
recurrence is strongly contractive; end-to-end output error ~1.5e-4
relative vs the fp32 reference.

Column interleave: state column s lives in col-group j=(s//32)%4 at free
offset 32*(s//128) + s%32. The (128, 512) psum holding state' (batch on
partitions within each 32-group) turns into the next step's stationary
operand layout via DVE 32x32 block-transposes: block (j, m') lands at
partitions [32j:32j+32] of k-tile m' -- exactly where matmul round m'
reads it.

Pipelining (the point of this version): each step's 17 k-rounds are
split into lo (psum cols 0:256 = k-tiles 0..7 of the next x) and hi
(cols 256:512 = k-tiles 8..15) accumulation groups. The lo group stops
half way through the step, so ACT tanh + DVE transpose of the lo chunks
overlap the hi matmuls, and the hi chunks' tanh/transpose overlaps the
next step's early rounds (which consume y/u/lo-chunk operands first).
The PE never waits on the full tanh->transpose chain. A tiles are
DMA-streamed in first-use order so step 0 starts as soon as tile k=0
lands; outputs are DMA'd out in slices as they are produced.
"""

import sys

for _p in ("/opt/trn_rl_repo", "/root/.axon_site/_ro/trn_rl_repo"):
    if _p not in sys.path:
        sys.path.insert(0, _p)

import numpy as np

N_STATE = 2048
N_U = 64
N_Y = 128
PAST = 64
FORE = 16
BATCH = 256
T = PAST + FORE          # 80 total steps; only 79 matmul steps needed
NSTEP = T - 1            # step t computes state_{t+1}; state_80 is unused
NK = 17                  # contraction tiles: 16 x 128 state + 1 x (64 u + 64 pad)
KDIM = NK * 128          # 2176 padded contraction size
N_CORES = 8
B = BATCH // N_CORES     # 32 per core
OUT_CHUNK = 10           # output DMA granularity in steps


def _build_program():
    import concourse.bass as bass
    import concourse.tile as tile
    from concourse import mybir

    F32 = mybir.dt.float32
    F16 = mybir.dt.float16

    nc = bass.Bass("TRN2", target_bir_lowering=False, debug=False,
                   num_devices=N_CORES)

    A_ext = nc.declare_dram_parameter("A_re", [KDIM, 4, 512], F16, isOutput=False)
    ytanhT_ext = nc.declare_dram_parameter("ytanhT", [128, PAST * B], F16, isOutput=False)
    utanhT_ext = nc.declare_dram_parameter("utanhT", [128, NSTEP * B], F16, isOutput=False)
    ywrap_ext = nc.declare_dram_parameter("ywrap", [128, (PAST - 1) * B], F32, isOutput=False)
    initxT_ext = nc.declare_dram_parameter("initxT", [128, 512], F16, isOutput=False)
    out_ext = nc.declare_dram_parameter("outbuf", [128, NSTEP * B], F32, isOutput=True)

    # psum slot permutation: slot p of the 16 32-col psum slots holds state
    # k-tile TILE_OF_SLOT[p]. Tiles 1..8 land in the lo half (produced and
    # tanh/transposed a half-phase early), tile 0 (the output/forecast tile)
    # leads the hi half, tiles 9..15 trail. Next step's k emission order puts
    # the statically-known y/u rounds and the early tiles 1..8 first, giving
    # the late chunks (9..15) ten rounds (~1.1us) of cover for their
    # tanh+transpose chain.
    TILE_OF_SLOT = [1, 2, 3, 4, 5, 6, 7, 8, 0, 9, 10, 11, 12, 13, 14, 15]
    SLOT_OF_TILE = [0] * 16
    for p, m in enumerate(TILE_OF_SLOT):
        SLOT_OF_TILE[m] = p
    korder_past = [0, 16] + list(range(1, 16))
    korder_fore = [16] + list(range(1, 16)) + [0]

    with tile.TileContext(nc) as tc:
        with tc.tile_pool(name="const", bufs=1) as cpool, \
             tc.tile_pool(name="xbuf", bufs=2) as xpool, \
             tc.tile_pool(name="th", bufs=2) as thpool, \
             tc.tile_pool(name="psum", bufs=2, space="PSUM") as pspool:

            ytanhT = cpool.tile([128, PAST * B], F16, tag="yt")
            nc.sync.dma_start(out=ytanhT[:], in_=ytanhT_ext[:])
            xT = xpool.tile([128, 512], F16, tag="xT")
            nc.sync.dma_start(out=xT[:], in_=initxT_ext[:])
            utanhT = cpool.tile([128, NSTEP * B], F16, tag="ut")
            nc.sync.dma_start(out=utanhT[:], in_=utanhT_ext[:])

            # A k-tiles as separate tiles, DMA'd in first-use order so each
            # round of step 0 waits only for its own tile.
            A_sb = [None] * NK
            for k in korder_past:
                A_sb[k] = cpool.tile([128, 2048], F16, tag=f"A{k}", name=f"A{k}")
                nc.sync.dma_start(out=A_sb[k][:],
                                  in_=A_ext[128 * k:128 * (k + 1), :, :])

            ywrap = cpool.tile([128, (PAST - 1) * B], F32, tag="yw")
            nc.sync.dma_start(out=ywrap[:], in_=ywrap_ext[:])
            outbuf = cpool.tile([128, NSTEP * B], F32, tag="ob")

            def lhs_for(t, k, x):
                if k == 0 and t < PAST:
                    return ytanhT[:, B * t:B * (t + 1)]
                if k == 16:
                    return utanhT[:, B * t:B * (t + 1)]
                p = SLOT_OF_TILE[k]
                return x[:, 32 * p:32 * (p + 1)]

            out_done = 0
            for t in range(NSTEP):
                korder = korder_past if t < PAST else korder_fore
                ps_lo = pspool.tile([128, 256], F32, tag="plo")
                ps_hi = pspool.tile([128, 256], F32, tag="phi")
                if t < NSTEP - 1:
                    nxT = xpool.tile([128, 512], F16, tag="xT", name="nxT")

                # the last step only needs output cols 0:128 (= psum slot 8 =
                # hi cols 0:32 of each quadrant); its lo half is never read.
                halves = ((0, ps_lo), (1, ps_hi)) if t < NSTEP - 1 else ((1, ps_hi),)
                for half, ps in halves:
                    c0 = 256 * half
                    for idx, k in enumerate(korder):
                        lhsT = lhs_for(t, k, xT)
                        start = idx == 0
                        stop = idx == len(korder) - 1
                        for j in range(4):
                            nc.tensor.matmul(
                                ps[32 * j:32 * (j + 1), :],
                                lhsT,
                                A_sb[k][:, 512 * j + c0:512 * j + c0 + 256],
                                start=start, stop=stop,
                                tile_position=(0, 32 * j),
                            )
                    if t == NSTEP - 1:
                        continue  # last state: only the output copy is needed
                    # tanh + 32x32 block transpose of this half's slots into
                    # the next step's stationary operand tile. Slot 8 (tile 0)
                    # is only needed once teacher forcing ends (t >= PAST-1).
                    if half == 0:
                        chunks = ((0, 128), (128, 256))
                    else:
                        hi0 = 32 if t < PAST - 1 else 0
                        chunks = ((hi0, 160), (160, 256))
                    for a, b_ in chunks:
                        th = thpool.tile([128, b_ - a], F16, tag=f"th{half}{a}")
                        nc.scalar.activation(th[:], ps[:, a:b_],
                                             mybir.ActivationFunctionType.Tanh)
                        nc.vector.transpose(nxT[:, c0 + a:c0 + b_], th[:])

                # output slot t (row t+1): expectation = psum slot 8 cols of
                # every partition group. GPSIMD can't read PSUM, so this rides
                # DVE, issued after the latency-critical transposes.
                if t + 1 < PAST:
                    nc.vector.tensor_sub(outbuf[:, B * t:B * (t + 1)],
                                         ps_hi[:, 0:32],
                                         ywrap[:, B * t:B * (t + 1)])
                else:
                    nc.vector.tensor_copy(outbuf[:, B * t:B * (t + 1)],
                                          ps_hi[:, 0:32])
                if t < NSTEP - 1:
                    xT = nxT

                # stream finished output slices out while compute continues
                if (t + 1) % OUT_CHUNK == 0:
                    nc.sync.dma_start(
                        out=out_ext[:, B * out_done:B * (t + 1)],
                        in_=outbuf[:, B * out_done:B * (t + 1)])
                    out_done = t + 1

            if out_done < NSTEP:
                nc.sync.dma_start(out=out_ext[:, B * out_done:],
                                  in_=outbuf[:, B * out_done:])

    _split_multi_waits(nc)
    return nc


def _split_multi_waits(nc):
    """This walrus build accepts at most one sem wait per instruction; Tile
    sometimes emits more. Hoist extras onto nops inserted just before the
    instruction in the same engine stream."""
    from concourse import mybir

    n = 0
    for f in nc.m.functions:
        for b in f.blocks:
            insts = b.instructions
            out = []
            changed = False
            for ins in insts:
                si = ins.sync_info
                if si is not None and len(si.on_wait) > 1:
                    waits = list(si.on_wait)
                    for w in waits[:-1]:
                        n += 1
                        out.append(mybir.InstNoOp(
                            name=f"I-waitsplit-{n}",
                            engine=ins.engine,
                            ins=[], outs=[],
                            bass_nofuse=True,
                            sync_info=mybir.SyncInfo(on_wait=[w], on_update=[]),
                        ))
                    ins.sync_info = mybir.SyncInfo(
                        on_wait=[waits[-1]], on_update=list(si.on_update))
                    changed = True
                out.append(ins)
            if changed:
                b.instructions = out


def _host_inputs(U, Y, A, init_state):
    """Build the per-core input maps (all pre-tanh / pre-transpose work)."""
    A = np.asarray(A, np.float32)
    U = np.asarray(U, np.float32)
    Y = np.asarray(Y, np.float32)
    init_state = np.asarray(init_state, np.float32)

    # psum slot permutation (must match _build_program's TILE_OF_SLOT)
    TILE_OF_SLOT = [1, 2, 3, 4, 5, 6, 7, 8, 0, 9, 10, 11, 12, 13, 14, 15]

    A_pad = np.zeros((KDIM, N_STATE), np.float16)
    A_pad[:N_STATE + N_U] = A.astype(np.float16)
    # column interleave: state col s (tile m=s//128, c=s%128) lands in
    # quadrant j=c//32 at free offset 32*slot(m) + c%32
    A_re = np.ascontiguousarray(
        A_pad.reshape(KDIM, 16, 4, 32).transpose(0, 2, 1, 3)[:, :, TILE_OF_SLOT, :]
        .reshape(KDIM, 4, 512))

    init_tanh = np.tanh(init_state[0]).astype(np.float16)          # (2048,)
    it = init_tanh.reshape(16, 128).T                              # (c, m)
    initxT = np.ascontiguousarray(
        np.broadcast_to(it[:, TILE_OF_SLOT][:, :, None],
                        (128, 16, 32)).reshape(128, 512))

    ytanh = np.tanh(Y).astype(np.float16)                          # (64, 256, 128)
    utanh = np.tanh(U[:NSTEP]).astype(np.float16)                  # (79, 256, 64)

    in_maps = []
    for c in range(N_CORES):
        b0 = c * B
        yt = np.ascontiguousarray(
            ytanh[:, b0:b0 + B, :].transpose(0, 2, 1)              # (64, 128, 32)
            .transpose(1, 0, 2).reshape(128, PAST * B))
        ut = np.zeros((128, NSTEP * B), np.float16)
        ut[:N_U] = (utanh[:, b0:b0 + B, :].transpose(0, 2, 1)      # (79, 64, 32)
                    .transpose(1, 0, 2).reshape(N_U, NSTEP * B))
        # ywrap slot s (=1..63) at cols 32*(s-1): rows 32j+b = Y[s, b0+b, 32j+cc]
        yw = (Y[1:PAST, b0:b0 + B, :].reshape(PAST - 1, B, 4, 32)
              .transpose(0, 2, 1, 3)                               # (63, 4, 32b, 32cc)
              .reshape(PAST - 1, 128, 32)
              .transpose(1, 0, 2).reshape(128, (PAST - 1) * B))
        in_maps.append({
            "A_re": A_re,
            "ytanhT": yt,
            "utanhT": np.ascontiguousarray(ut),
            "ywrap": np.ascontiguousarray(yw.astype(np.float32)),
            "initxT": initxT,
        })
    return in_maps


def kernel(U, Y, A, init_state):
    from concourse.bass_utils import run_bass_kernel_spmd

    nc = _build_program()
    in_maps = _host_inputs(U, Y, A, init_state)
    res = run_bass_kernel_spmd(nc, in_maps, list(range(N_CORES)))

    out = np.empty((T, BATCH, N_Y), np.float32)
    # slot 0: err for t=0 is pure host math (state_0 = broadcast init_state)
    out[0] = np.asarray(init_state, np.float32)[0, :N_Y][None, :] - np.asarray(Y, np.float32)[0]
    for c in range(N_CORES):
        b0 = c * B
        ob = res.results[c]["outbuf"]                              # (128, 79*32)
        # [32j+b, 32t+cc] = out[t+1, b0+b, 32j+cc]
        ob4 = ob.reshape(4, 32, NSTEP, 32)                         # (j, b, t, cc)
        out[1:, b0:b0 + B, :] = ob4.transpose(2, 1, 0, 3).reshape(NSTEP, B, N_Y)
    return out


if __name__ == "__main__":
    rng = np.random.default_rng(0)
    U = rng.standard_normal((T, BATCH, N_U)).astype(np.float32)
    Y = rng.standard_normal((PAST, BATCH, N_Y)).astype(np.float32)
    A = (rng.standard_normal((N_STATE + N_U, N_STATE)) * 0.02).astype(np.float32)
    init = rng.standard_normal((1, N_STATE)).astype(np.float32)
    o = kernel(U=U, Y=Y, A=A, init_state=init)
    print("kernel out:", o.shape, o.dtype)


# revision 16
# speedup vs baseline: 1.1048x; 1.0017x over previous
"""HCNN (known-U) recurrence kernel for 8 Trainium2 NeuronCores.

Model (see reference): 80 sequential steps of
    state' = tanh(cat(post_state, u)) @ A            A: (2112, 2048) fp32
with teacher forcing post_state[:, :128] = y during the 64 past steps,
outputs = 64 past errors then 16 forecasts (first 128 state components).

Strategy
--------
Data-parallel over batch: 256 = 8 cores x 32. Each core runs the full
recurrence for its batch slice; no collectives.

Per-core per-step matmul x @ A with batch M=32 would waste 3/4 of the
128-wide PE array, so the A columns are split into 4 interleaved groups
and computed by 4 concurrent column-tiled matmuls (tile_position=(0,32j))
sharing the array. Data is fp16 (single pass): the teacher-forced
recurrence is strongly contractive; end-to-end output error ~1.5e-4
relative vs the fp32 reference.

Column interleave: state column s lives in col-group j=(s//32)%4 at free
offset 32*(s//128) + s%32. The (128, 512) psum holding state' (batch on
partitions within each 32-group) turns into the next step's stationary
operand layout via DVE 32x32 block-transposes: block (j, m') lands at
partitions [32j:32j+32] of k-tile m' -- exactly where matmul round m'
reads it.

Pipelining (the point of this version): each step's 17 k-rounds are
split into lo (psum cols 0:256 = k-tiles 0..7 of the next x) and hi
(cols 256:512 = k-tiles 8..15) accumulation groups. The lo group stops
half way through the step, so ACT tanh + DVE transpose of the lo chunks
overlap the hi matmuls, and the hi chunks' tanh/transpose overlaps the
next step's early rounds (which consume y/u/lo-chunk operands first).
The PE never waits on the full tanh->transpose chain. A tiles are
DMA-streamed in first-use order so step 0 starts as soon as tile k=0
lands; outputs are DMA'd out in slices as they are produced.
"""

import sys

for _p in ("/opt/trn_rl_repo", "/root/.axon_site/_ro/trn_rl_repo"):
    if _p not in sys.path:
        sys.path.insert(0, _p)

import numpy as np

N_STATE = 2048
N_U = 64
N_Y = 128
PAST = 64
FORE = 16
BATCH = 256
T = PAST + FORE          # 80 total steps; only 79 matmul steps needed
NSTEP = T - 1            # step t computes state_{t+1}; state_80 is unused
NK = 17                  # contraction tiles: 16 x 128 state + 1 x (64 u + 64 pad)
KDIM = NK * 128          # 2176 padded contraction size
N_CORES = 8
B = BATCH // N_CORES     # 32 per core
OUT_CHUNK = 10           # output DMA granularity in steps

# psum slot permutation: slot p of the 16 32-col psum slots holds state
# k-tile TILE_OF_SLOT[p]. Tiles 1..9 form the lo accumulation group (cols
# 0:288); tile 0 (the output/forecast tile) leads the hi group, 10..15 trail.
TILE_OF_SLOT = [1, 2, 3, 4, 5, 6, 7, 8, 9, 0, 10, 11, 12, 13, 14, 15]
SLOT_OF_TILE = [0] * 16
for _p, _m in enumerate(TILE_OF_SLOT):
    SLOT_OF_TILE[_m] = _p
LO = 288                 # lo accumulation group column count


def _build_program():
    import concourse.bass as bass
    import concourse.tile as tile
    from concourse import mybir

    F32 = mybir.dt.float32
    F16 = mybir.dt.float16

    nc = bass.Bass("TRN2", target_bir_lowering=False, debug=False,
                   num_devices=N_CORES)

    A_ext = nc.declare_dram_parameter("A_re", [KDIM, 4, 512], F16, isOutput=False)
    ytanhT_ext = nc.declare_dram_parameter("ytanhT", [128, PAST * B], F16, isOutput=False)
    utanhT_ext = nc.declare_dram_parameter("utanhT", [128, NSTEP * B], F16, isOutput=False)
    ywrap_ext = nc.declare_dram_parameter("ywrap", [128, (PAST - 1) * B], F32, isOutput=False)
    initxT_ext = nc.declare_dram_parameter("initxT", [128, 512], F16, isOutput=False)
    out_ext = nc.declare_dram_parameter("outbuf", [128, NSTEP * B], F32, isOutput=True)

    # The lo group's tiles are produced and tanh/transposed a half-phase
    # early. Next step's k emission order puts the statically-known y/u
    # rounds and the early tiles first, giving the late hi chunks ~1.4us of
    # cover for the stop->tanh->transpose chain (~1.2us with 64-col chunks).
    korder_past = [0, 16] + list(range(1, 16))
    korder_fore = [16] + list(range(1, 10)) + [0] + list(range(10, 16))

    with tile.TileContext(nc) as tc:
        with tc.tile_pool(name="const", bufs=1) as cpool, \
             tc.tile_pool(name="xbuf", bufs=2) as xpool, \
             tc.tile_pool(name="th", bufs=2) as thpool, \
             tc.tile_pool(name="psum", bufs=2, space="PSUM") as pspool:

            ytanhT = cpool.tile([128, PAST * B], F16, tag="yt")
            nc.sync.dma_start(out=ytanhT[:], in_=ytanhT_ext[:])
            xT = xpool.tile([128, 512], F16, tag="xT")
            nc.sync.dma_start(out=xT[:], in_=initxT_ext[:])
            utanhT = cpool.tile([128, NSTEP * B], F16, tag="ut")
            nc.sync.dma_start(out=utanhT[:], in_=utanhT_ext[:])

            # A k-tiles as separate tiles, DMA'd in first-use order so each
            # round of step 0 waits only for its own tile.
            A_sb = [None] * NK
            for k in korder_past:
                A_sb[k] = cpool.tile([128, 2048], F16, tag=f"A{k}", name=f"A{k}")
                nc.sync.dma_start(out=A_sb[k][:],
                                  in_=A_ext[128 * k:128 * (k + 1), :, :])

            ywrap = cpool.tile([128, (PAST - 1) * B], F32, tag="yw")
            nc.sync.dma_start(out=ywrap[:], in_=ywrap_ext[:])
            outbuf = cpool.tile([128, NSTEP * B], F32, tag="ob")

            def lhs_for(t, k, x):
                if k == 0 and t < PAST:
                    return ytanhT[:, B * t:B * (t + 1)]
                if k == 16:
                    return utanhT[:, B * t:B * (t + 1)]
                p = SLOT_OF_TILE[k]
                return x[:, 32 * p:32 * (p + 1)]

            out_done = 0
            for t in range(NSTEP):
                korder = korder_past if t < PAST else korder_fore
                # full-bank psum tiles so the two accumulation groups never
                # share a zero region; only the leading LO / 512-LO columns
                # are used.
                ps_lo = pspool.tile([128, 512], F32, tag="plo")
                ps_hi = pspool.tile([128, 512], F32, tag="phi")
                if t < NSTEP - 1:
                    nxT = xpool.tile([128, 512], F16, tag="xT", name="nxT")

                # the last step only needs output cols 0:128 (= psum slot 9 =
                # hi cols 0:32 of each quadrant); its lo half is never read.
                halves = ((0, ps_lo), (1, ps_hi)) if t < NSTEP - 1 else ((1, ps_hi),)
                for half, ps in halves:
                    c0, w = (0, LO) if half == 0 else (LO, 512 - LO)
                    for idx, k in enumerate(korder):
                        lhsT = lhs_for(t, k, xT)
                        start = idx == 0
                        stop = idx == len(korder) - 1
                        for j in range(4):
                            nc.tensor.matmul(
                                ps[32 * j:32 * (j + 1), 0:w],
                                lhsT,
                                A_sb[k][:, 512 * j + c0:512 * j + c0 + w],
                                start=start, stop=stop,
                                tile_position=(0, 32 * j),
                            )
                    if t == NSTEP - 1:
                        continue  # last state: only the output copy is needed
                    # tanh + 32x32 block transpose of this half's slots into
                    # the next step's stationary operand tile. The hi chunks
                    # are on the next step's critical path: keep them 64 cols
                    # for low latency. Slot 9 (tile 0) is only needed once
                    # teacher forcing ends (t >= PAST-1).
                    if half == 0:
                        chunks = ((0, 128), (128, 288))
                    elif t < PAST - 1:
                        chunks = ((320, 384), (384, 448), (448, 512))
                    else:
                        chunks = ((288, 352), (352, 416), (416, 480), (480, 512))
                    for a, b_ in chunks:
                        th = thpool.tile([128, b_ - a], F16, tag=f"th{half}{a}")
                        nc.scalar.activation(th[:], ps[:, a - c0:b_ - c0],
                                             mybir.ActivationFunctionType.Tanh)
                        nc.vector.transpose(nxT[:, a:b_], th[:])

                # output slot t (row t+1): expectation = psum slot 8 cols of
                # every partition group. GPSIMD can't read PSUM, so this rides
                # DVE, issued after the latency-critical transposes.
                if t + 1 < PAST:
                    nc.vector.tensor_sub(outbuf[:, B * t:B * (t + 1)],
                                         ps_hi[:, 0:32],
                                         ywrap[:, B * t:B * (t + 1)])
                else:
                    nc.vector.tensor_copy(outbuf[:, B * t:B * (t + 1)],
                                          ps_hi[:, 0:32])
                if t < NSTEP - 1:
                    xT = nxT

                # stream finished output slices out while compute continues
                if (t + 1) % OUT_CHUNK == 0:
                    nc.sync.dma_start(
                        out=out_ext[:, B * out_done:B * (t + 1)],
                        in_=outbuf[:, B * out_done:B * (t + 1)])
                    out_done = t + 1

            if out_done < NSTEP:
                nc.sync.dma_start(out=out_ext[:, B * out_done:],
                                  in_=outbuf[:, B * out_done:])

    _split_multi_waits(nc)
    return nc


def _split_multi_waits(nc):
    """This walrus build accepts at most one sem wait per instruction; Tile
    sometimes emits more. Hoist extras onto nops inserted just before the
    instruction in the same engine stream."""
    from concourse import mybir

    n = 0
    for f in nc.m.functions:
        for b in f.blocks:
            insts = b.instructions
            out = []
            changed = False
            for ins in insts:
                si = ins.sync_info
                if si is not None and len(si.on_wait) > 1:
                    waits = list(si.on_wait)
                    for w in waits[:-1]:
                        n += 1
                        out.append(mybir.InstNoOp(
                            name=f"I-waitsplit-{n}",
                            engine=ins.engine,
                            ins=[], outs=[],
                            bass_nofuse=True,
                            sync_info=mybir.SyncInfo(on_wait=[w], on_update=[]),
                        ))
                    ins.sync_info = mybir.SyncInfo(
                        on_wait=[waits[-1]], on_update=list(si.on_update))
                    changed = True
                out.append(ins)
            if changed:
                b.instructions = out


def _host_inputs(U, Y, A, init_state):
    """Build the per-core input maps (all pre-tanh / pre-transpose work)."""
    A = np.asarray(A, np.float32)
    U = np.asarray(U, np.float32)
    Y = np.asarray(Y, np.float32)
    init_state = np.asarray(init_state, np.float32)

    A_pad = np.zeros((KDIM, N_STATE), np.float16)
    A_pad[:N_STATE + N_U] = A.astype(np.float16)
    # column interleave: state col s (tile m=s//128, c=s%128) lands in
    # quadrant j=c//32 at free offset 32*slot(m) + c%32
    A_re = np.ascontiguousarray(
        A_pad.reshape(KDIM, 16, 4, 32).transpose(0, 2, 1, 3)[:, :, TILE_OF_SLOT, :]
        .reshape(KDIM, 4, 512))

    init_tanh = np.tanh(init_state[0]).astype(np.float16)          # (2048,)
    it = init_tanh.reshape(16, 128).T                              # (c, m)
    initxT = np.ascontiguousarray(
        np.broadcast_to(it[:, TILE_OF_SLOT][:, :, None],
                        (128, 16, 32)).reshape(128, 512))

    ytanh = np.tanh(Y).astype(np.float16)                          # (64, 256, 128)
    utanh = np.tanh(U[:NSTEP]).astype(np.float16)                  # (79, 256, 64)

    in_maps = []
    for c in range(N_CORES):
        b0 = c * B
        yt = np.ascontiguousarray(
            ytanh[:, b0:b0 + B, :].transpose(0, 2, 1)              # (64, 128, 32)
            .transpose(1, 0, 2).reshape(128, PAST * B))
        ut = np.zeros((128, NSTEP * B), np.float16)
        ut[:N_U] = (utanh[:, b0:b0 + B, :].transpose(0, 2, 1)      # (79, 64, 32)
                    .transpose(1, 0, 2).reshape(N_U, NSTEP * B))
        # ywrap slot s (=1..63) at cols 32*(s-1): rows 32j+b = Y[s, b0+b, 32j+cc]
        yw = (Y[1:PAST, b0:b0 + B, :].reshape(PAST - 1, B, 4, 32)
              .transpose(0, 2, 1, 3)                               # (63, 4, 32b, 32cc)
              .reshape(PAST - 1, 128, 32)
              .transpose(1, 0, 2).reshape(128, (PAST - 1) * B))
        in_maps.append({
            "A_re": A_re,
            "ytanhT": yt,
            "utanhT": np.ascontiguousarray(ut),
            "ywrap": np.ascontiguousarray(yw.astype(np.float32)),
            "initxT": initxT,
        })
    return in_maps


def kernel(U, Y, A, init_state):
    from concourse.bass_utils import run_bass_kernel_spmd

    nc = _build_program()
    in_maps = _host_inputs(U, Y, A, init_state)
    res = run_bass_kernel_spmd(nc, in_maps, list(range(N_CORES)))

    out = np.empty((T, BATCH, N_Y), np.float32)
    # slot 0: err for t=0 is pure host math (state_0 = broadcast init_state)
    out[0] = np.asarray(init_state, np.float32)[0, :N_Y][None, :] - np.asarray(Y, np.float32)[0]
    for c in range(N_CORES):
        b0 = c * B
        ob = res.results[c]["outbuf"]                              # (128, 79*32)
        # [32j+b, 32t+cc] = out[t+1, b0+b, 32j+cc]
        ob4 = ob.reshape(4, 32, NSTEP, 32)                         # (j, b, t, cc)
        out[1:, b0:b0 + B, :] = ob4.transpose(2, 1, 0, 3).reshape(NSTEP, B, N_Y)
    return out


if __name__ == "__main__":
    rng = np.random.default_rng(0)
    U = rng.standard_normal((T, BATCH, N_U)).astype(np.float32)
    Y = rng.standard_normal((PAST, BATCH, N_Y)).astype(np.float32)
    A = (rng.standard_normal((N_STATE + N_U, N_STATE)) * 0.02).astype(np.float32)
    init = rng.standard_normal((1, N_STATE)).astype(np.float32)
    o = kernel(U=U, Y=Y, A=A, init_state=init)
    print("kernel out:", o.shape, o.dtype)


# revision 17
# speedup vs baseline: 1.2345x; 1.1174x over previous
"""HCNN (known-U) recurrence kernel for 8 Trainium2 NeuronCores.

Model (see reference): 80 sequential steps of
    state' = tanh(cat(post_state, u)) @ A            A: (2112, 2048) fp32
with teacher forcing post_state[:, :128] = y during the 64 past steps,
outputs = 64 past errors then 16 forecasts (first 128 state components).

Strategy
--------
Data-parallel over batch: 256 = 8 cores x 32. Each core runs the full
recurrence for its batch slice; no collectives.

Per-core per-step matmul x @ A with batch M=32 would waste 3/4 of the
128-wide PE array, so the A columns are split into 4 interleaved groups
and computed by 4 concurrent column-tiled matmuls (tile_position=(0,32j))
sharing the array. Data is fp16 (single pass): the teacher-forced
recurrence is strongly contractive; end-to-end output error ~1.5e-4
relative vs the fp32 reference.

Column interleave: state column s lives in col-group j=(s//32)%4 at free
offset 32*(s//128) + s%32. The (128, 512) psum holding state' (batch on
partitions within each 32-group) turns into the next step's stationary
operand layout via DVE 32x32 block-transposes: block (j, m') lands at
partitions [32j:32j+32] of k-tile m' -- exactly where matmul round m'
reads it.

Pipelining (the point of this version): each step's 17 k-rounds are
split into lo (psum cols 0:256 = k-tiles 0..7 of the next x) and hi
(cols 256:512 = k-tiles 8..15) accumulation groups. The lo group stops
half way through the step, so ACT tanh + DVE transpose of the lo chunks
overlap the hi matmuls, and the hi chunks' tanh/transpose overlaps the
next step's early rounds (which consume y/u/lo-chunk operands first).
The PE never waits on the full tanh->transpose chain. A tiles are
DMA-streamed in first-use order so step 0 starts as soon as tile k=0
lands; outputs are DMA'd out in slices as they are produced.
"""

import sys

for _p in ("/opt/trn_rl_repo", "/root/.axon_site/_ro/trn_rl_repo"):
    if _p not in sys.path:
        sys.path.insert(0, _p)

import numpy as np

N_STATE = 2048
N_U = 64
N_Y = 128
PAST = 64
FORE = 16
BATCH = 256
T = PAST + FORE          # 80 total steps; only 79 matmul steps needed
NSTEP = T - 1            # step t computes state_{t+1}; state_80 is unused
NK = 17                  # contraction tiles: 16 x 128 state + 1 x (64 u + 64 pad)
KDIM = NK * 128          # 2176 padded contraction size
N_CORES = 8
B = BATCH // N_CORES     # 32 per core
OUT_CHUNK = 10           # output DMA granularity in steps

# psum slot permutation: slot p of the 16 32-col psum slots holds state
# k-tile TILE_OF_SLOT[p]. Tiles 1..9 form the lo accumulation group (cols
# 0:288); tile 0 (the output/forecast tile) leads the hi group, 10..15 trail.
TILE_OF_SLOT = [1, 2, 3, 4, 5, 6, 7, 8, 9, 0, 10, 11, 12, 13, 14, 15]
SLOT_OF_TILE = [0] * 16
for _p, _m in enumerate(TILE_OF_SLOT):
    SLOT_OF_TILE[_m] = _p
LO = 288                 # lo accumulation group column count


def _build_program():
    import concourse.bass as bass
    import concourse.tile as tile
    from concourse import mybir

    F32 = mybir.dt.float32
    F16 = mybir.dt.float16

    nc = bass.Bass("TRN2", target_bir_lowering=False, debug=False,
                   num_devices=N_CORES)

    A_ext = nc.declare_dram_parameter("A_re", [KDIM, 4, 512], F16, isOutput=False)
    ytanhT_ext = nc.declare_dram_parameter("ytanhT", [128, PAST * B], F16, isOutput=False)
    utanhT_ext = nc.declare_dram_parameter("utanhT", [128, NSTEP * B], F16, isOutput=False)
    ywrap_ext = nc.declare_dram_parameter("ywrap", [128, (PAST - 1) * B], F32, isOutput=False)
    initxT_ext = nc.declare_dram_parameter("initxT", [128, 512], F16, isOutput=False)
    out_ext = nc.declare_dram_parameter("outbuf", [128, NSTEP * B], F32, isOutput=True)

    # The lo group's tiles are produced and tanh/transposed a half-phase
    # early. Next step's k emission order puts the statically-known y/u
    # rounds and the early tiles first, giving the late hi chunks ~1.4us of
    # cover for the stop->tanh->transpose chain (~1.2us with 64-col chunks).
    korder_past = [0, 16] + list(range(1, 16))
    korder_fore = [16] + list(range(1, 10)) + [0] + list(range(10, 16))

    with tile.TileContext(nc) as tc:
        with tc.tile_pool(name="const", bufs=1) as cpool, \
             tc.tile_pool(name="xbuf", bufs=2) as xpool, \
             tc.tile_pool(name="th", bufs=2) as thpool, \
             tc.tile_pool(name="psum", bufs=2, space="PSUM") as pspool:

            ytanhT = cpool.tile([128, PAST * B], F16, tag="yt")
            nc.sync.dma_start(out=ytanhT[:], in_=ytanhT_ext[:])
            xT = xpool.tile([128, 512], F16, tag="xT")
            nc.sync.dma_start(out=xT[:], in_=initxT_ext[:])
            utanhT = cpool.tile([128, NSTEP * B], F16, tag="ut")
            nc.sync.dma_start(out=utanhT[:], in_=utanhT_ext[:])

            # A k-tiles as separate tiles, DMA'd in first-use order so each
            # round of step 0 waits only for its own tile.
            A_sb = [None] * NK
            for k in korder_past:
                A_sb[k] = cpool.tile([128, 2048], F16, tag=f"A{k}", name=f"A{k}")
                nc.sync.dma_start(out=A_sb[k][:],
                                  in_=A_ext[128 * k:128 * (k + 1), :, :])

            ywrap = cpool.tile([128, (PAST - 1) * B], F32, tag="yw")
            nc.sync.dma_start(out=ywrap[:], in_=ywrap_ext[:])
            outbuf = cpool.tile([128, NSTEP * B], F32, tag="ob")

            def lhs_for(t, k, x):
                if k == 0 and t < PAST:
                    return ytanhT[:, B * t:B * (t + 1)]
                if k == 16:
                    return utanhT[:, B * t:B * (t + 1)]
                p = SLOT_OF_TILE[k]
                return x[:, 32 * p:32 * (p + 1)]

            out_done = 0
            for t in range(NSTEP):
                korder = korder_past if t < PAST else korder_fore
                # full-bank psum tiles so the two accumulation groups never
                # share a zero region; only the leading LO / 512-LO columns
                # are used.
                ps_lo = pspool.tile([128, 512], F32, tag="plo")
                ps_hi = pspool.tile([128, 512], F32, tag="phi")
                if t < NSTEP - 1:
                    nxT = xpool.tile([128, 512], F16, tag="xT", name="nxT")

                # the last step only needs output cols 0:128 (= psum slot 9 =
                # hi cols 0:32 of each quadrant); its lo half is never read.
                halves = ((0, ps_lo), (1, ps_hi)) if t < NSTEP - 1 else ((1, ps_hi),)
                for half, ps in halves:
                    c0, w = (0, LO) if half == 0 else (LO, 512 - LO)
                    for idx, k in enumerate(korder):
                        lhsT = lhs_for(t, k, xT)
                        start = idx == 0
                        stop = idx == len(korder) - 1
                        for j in range(4):
                            nc.tensor.matmul(
                                ps[32 * j:32 * (j + 1), 0:w],
                                lhsT,
                                A_sb[k][:, 512 * j + c0:512 * j + c0 + w],
                                start=start, stop=stop,
                                tile_position=(0, 32 * j),
                            )
                    if t == NSTEP - 1:
                        continue  # last state: only the output copy is needed
                    # tanh + 32x32 block transpose of this half's slots into
                    # the next step's stationary operand tile. The hi chunks
                    # are on the next step's critical path: keep them 64 cols
                    # for low latency. Slot 9 (tile 0) is only needed once
                    # teacher forcing ends (t >= PAST-1).
                    if half == 0:
                        chunks = ((0, 128), (128, 288))
                    elif t < PAST - 1:
                        chunks = ((320, 384), (384, 448), (448, 512))
                    else:
                        chunks = ((288, 352), (352, 416), (416, 480), (480, 512))
                    for a, b_ in chunks:
                        th = thpool.tile([128, b_ - a], F16, tag=f"th{half}{a}")
                        nc.scalar.activation(th[:], ps[:, a - c0:b_ - c0],
                                             mybir.ActivationFunctionType.Tanh)
                        nc.vector.transpose(nxT[:, a:b_], th[:])

                # output slot t (row t+1): expectation = psum slot 8 cols of
                # every partition group. GPSIMD can't read PSUM, so this rides
                # DVE, issued after the latency-critical transposes.
                if t + 1 < PAST:
                    nc.vector.tensor_sub(outbuf[:, B * t:B * (t + 1)],
                                         ps_hi[:, 0:32],
                                         ywrap[:, B * t:B * (t + 1)])
                else:
                    nc.vector.tensor_copy(outbuf[:, B * t:B * (t + 1)],
                                          ps_hi[:, 0:32])
                if t < NSTEP - 1:
                    xT = nxT

                # stream finished output slices out while compute continues
                if (t + 1) % OUT_CHUNK == 0:
                    nc.sync.dma_start(
                        out=out_ext[:, B * out_done:B * (t + 1)],
                        in_=outbuf[:, B * out_done:B * (t + 1)])
                    out_done = t + 1

            if out_done < NSTEP:
                nc.sync.dma_start(out=out_ext[:, B * out_done:],
                                  in_=outbuf[:, B * out_done:])

    _thin_matmul_sems(nc)
    _split_multi_waits(nc)
    return nc


def _thin_matmul_sems(nc):
    """Every matmul carries '++@complete' on the PE counting semaphore, and an
    instruction with semaphore ops costs ~34ns of PE sequencer time vs ~3ns
    without -- at 136 matmuls/step the sequencer, not the PE array, ends up
    pacing the kernel (~136ns/round floor). All waits on that semaphore sit
    exactly at stop-round boundaries, so only the 4 stop matmuls of each
    accumulation group need their updates: strip the rest and renumber every
    wait threshold from all-matmul counts to kept-update counts."""
    import dataclasses

    sem_ids = set()
    for f in nc.m.functions:
        for b in f.blocks:
            for ins in b.instructions:
                if type(ins).__name__ == "InstMatmult" and ins.sync_info:
                    for u in ins.sync_info.on_update:
                        sem_ids.add(u.id)
    if not sem_ids:
        return
    assert len(sem_ids) == 1, sem_ids
    sem = sem_ids.pop()

    mm_count = 0
    kept = 0
    remap = {}
    for f in nc.m.functions:
        for b in f.blocks:
            for ins in b.instructions:
                if type(ins).__name__ != "InstMatmult":
                    continue
                mm_count += 1
                si = ins.sync_info
                if ins.stop_tensor_calc:
                    kept += 1
                    remap[mm_count] = kept
                elif si is not None and si.on_update:
                    from concourse import mybir
                    ins.sync_info = mybir.SyncInfo(
                        on_wait=list(si.on_wait), on_update=[])

    for f in nc.m.functions:
        for b in f.blocks:
            for ins in b.instructions:
                si = ins.sync_info
                if si is None or not si.on_wait:
                    continue
                changed = False
                new_waits = []
                for w in si.on_wait:
                    if w.id == sem:
                        assert w.wait_value in remap, (
                            f"wait on PE sem at non-stop boundary: {w}")
                        new_waits.append(
                            dataclasses.replace(w, wait_value=remap[w.wait_value]))
                        changed = True
                    else:
                        new_waits.append(w)
                if changed:
                    from concourse import mybir
                    ins.sync_info = mybir.SyncInfo(
                        on_wait=new_waits, on_update=list(si.on_update))


def _split_multi_waits(nc):
    """This walrus build accepts at most one sem wait per instruction; Tile
    sometimes emits more. Hoist extras onto nops inserted just before the
    instruction in the same engine stream."""
    from concourse import mybir

    n = 0
    for f in nc.m.functions:
        for b in f.blocks:
            insts = b.instructions
            out = []
            changed = False
            for ins in insts:
                si = ins.sync_info
                if si is not None and len(si.on_wait) > 1:
                    waits = list(si.on_wait)
                    for w in waits[:-1]:
                        n += 1
                        out.append(mybir.InstNoOp(
                            name=f"I-waitsplit-{n}",
                            engine=ins.engine,
                            ins=[], outs=[],
                            bass_nofuse=True,
                            sync_info=mybir.SyncInfo(on_wait=[w], on_update=[]),
                        ))
                    ins.sync_info = mybir.SyncInfo(
                        on_wait=[waits[-1]], on_update=list(si.on_update))
                    changed = True
                out.append(ins)
            if changed:
                b.instructions = out


def _host_inputs(U, Y, A, init_state):
    """Build the per-core input maps (all pre-tanh / pre-transpose work)."""
    A = np.asarray(A, np.float32)
    U = np.asarray(U, np.float32)
    Y = np.asarray(Y, np.float32)
    init_state = np.asarray(init_state, np.float32)

    A_pad = np.zeros((KDIM, N_STATE), np.float16)
    A_pad[:N_STATE + N_U] = A.astype(np.float16)
    # column interleave: state col s (tile m=s//128, c=s%128) lands in
    # quadrant j=c//32 at free offset 32*slot(m) + c%32
    A_re = np.ascontiguousarray(
        A_pad.reshape(KDIM, 16, 4, 32).transpose(0, 2, 1, 3)[:, :, TILE_OF_SLOT, :]
        .reshape(KDIM, 4, 512))

    init_tanh = np.tanh(init_state[0]).astype(np.float16)          # (2048,)
    it = init_tanh.reshape(16, 128).T                              # (c, m)
    initxT = np.ascontiguousarray(
        np.broadcast_to(it[:, TILE_OF_SLOT][:, :, None],
                        (128, 16, 32)).reshape(128, 512))

    ytanh = np.tanh(Y).astype(np.float16)                          # (64, 256, 128)
    utanh = np.tanh(U[:NSTEP]).astype(np.float16)                  # (79, 256, 64)

    in_maps = []
    for c in range(N_CORES):
        b0 = c * B
        yt = np.ascontiguousarray(
            ytanh[:, b0:b0 + B, :].transpose(0, 2, 1)              # (64, 128, 32)
            .transpose(1, 0, 2).reshape(128, PAST * B))
        ut = np.zeros((128, NSTEP * B), np.float16)
        ut[:N_U] = (utanh[:, b0:b0 + B, :].transpose(0, 2, 1)      # (79, 64, 32)
                    .transpose(1, 0, 2).reshape(N_U, NSTEP * B))
        # ywrap slot s (=1..63) at cols 32*(s-1): rows 32j+b = Y[s, b0+b, 32j+cc]
        yw = (Y[1:PAST, b0:b0 + B, :].reshape(PAST - 1, B, 4, 32)
              .transpose(0, 2, 1, 3)                               # (63, 4, 32b, 32cc)
              .reshape(PAST - 1, 128, 32)
              .transpose(1, 0, 2).reshape(128, (PAST - 1) * B))
        in_maps.append({
            "A_re": A_re,
            "ytanhT": yt,
            "utanhT": np.ascontiguousarray(ut),
            "ywrap": np.ascontiguousarray(yw.astype(np.float32)),
            "initxT": initxT,
        })
    return in_maps


def kernel(U, Y, A, init_state):
    from concourse.bass_utils import run_bass_kernel_spmd

    nc = _build_program()
    in_maps = _host_inputs(U, Y, A, init_state)
    res = run_bass_kernel_spmd(nc, in_maps, list(range(N_CORES)))

    out = np.empty((T, BATCH, N_Y), np.float32)
    # slot 0: err for t=0 is pure host math (state_0 = broadcast init_state)
    out[0] = np.asarray(init_state, np.float32)[0, :N_Y][None, :] - np.asarray(Y, np.float32)[0]
    for c in range(N_CORES):
        b0 = c * B
        ob = res.results[c]["outbuf"]                              # (128, 79*32)
        # [32j+b, 32t+cc] = out[t+1, b0+b, 32j+cc]
        ob4 = ob.reshape(4, 32, NSTEP, 32)                         # (j, b, t, cc)
        out[1:, b0:b0 + B, :] = ob4.transpose(2, 1, 0, 3).reshape(NSTEP, B, N_Y)
    return out


if __name__ == "__main__":
    rng = np.random.default_rng(0)
    U = rng.standard_normal((T, BATCH, N_U)).astype(np.float32)
    Y = rng.standard_normal((PAST, BATCH, N_Y)).astype(np.float32)
    A = (rng.standard_normal((N_STATE + N_U, N_STATE)) * 0.02).astype(np.float32)
    init = rng.standard_normal((1, N_STATE)).astype(np.float32)
    o = kernel(U=U, Y=Y, A=A, init_state=init)
    print("kernel out:", o.shape, o.dtype)
